# revision 22
# baseline (speedup 1.0000x reference)
"""Trainium2 Bass kernel for nn_CNN_Att_16887811408068.

Self-contained: hardcodes shapes/sharding. Data-parallel over batch on
8 NeuronCores. Each core gets a privately remapped embedding table
(its <=16K distinct tokens packed into [32768, 128] f16), so SWDGE
transpose-mode gathers (int16 indices, 256B elements) land embeddings
directly in [dims x tokens] layout -- no PE transposes, no parity
selects. Global rows are padded to 448 tokens and gathered two rows per
896-index gather (the SWDGE descriptor-ring maximum), round-robin over
2 SWDGE queues (queue parity == semaphore-lane parity keeps per-lane
completion FIFO). The batch min/max "comparison" needs only the last 4
positions at this tolerance; its tiny AllGather is issued right after
the tail scores so it overlaps the conv stream (tile_wait_until staging
keeps the scheduler from queueing it behind the gathers).
"""
import numpy as np

import concourse.bacc as bacc
import concourse.mybir as mybir
import concourse.tile as tile
from concourse.ap import AP
from concourse.bass_utils import run_bass_kernel_spmd

F32 = mybir.dt.float32
F16 = mybir.dt.float16
I16 = mybir.dt.int16
AF = mybir.ActivationFunctionType
ALU = mybir.AluOpType
AX = mybir.AxisListType

# problem constants
V, D, WIN, P, CR = 50000, 100, 5, 411, 0.8
LOCAL, NF, GOUT, NCLS = 100, 100, 100, 2987
B, LL, LG = 256, P + WIN - 1, 411
NCORE = 8
BSH = B // NCORE                     # 32 batch rows per core

# local tail: positions p in [P0, P); token range t in [P0, LL)
NP_TAIL = 4
P0 = P - NP_TAIL                     # 407
TBLK = LL - P0                       # 8 token blocks (t values)
NTAIL = TBLK * BSH                   # 256 tail token slots
JOFF = 2 * BSH                       # col offset of t'=P0+2 block
NJ = NP_TAIL * BSH                   # 128 judge cols

# global path
TOKR = 448                           # padded tokens per row
GCH = 2                              # rows per gather chunk
NGI = GCH * TOKR                     # 896 indices per gather
NCHUNK = BSH // GCH                  # 16 chunks
VT = 32768                           # remapped table rows
NQ = 2                               # SWDGE queues

N_TILES_OUT = [(i * 512, min(512, NCLS - i * 512))
               for i in range((NCLS + 511) // 512)]

_CACHE = {}


def _wrap_idx(vals):
    """int16 gather index layout: idx i at [i%16, i//16], replicated to
    all eight 16-partition groups -> [128, n//16]."""
    n = len(vals)
    g = np.zeros((16, n // 16), np.int16)
    g[np.arange(n) % 16, np.arange(n) // 16] = vals.astype(np.int16)
    return np.tile(g, (8, 1))


def _win_ap(t, col0, n):
    """overlapping window view [1, n, WIN] starting at free col col0."""
    base = t[0:1, col0:col0 + n]
    return AP(base.tensor, base.offset, [list(base.ap[0]), [1, n], [1, WIN]])


def _build():
    nc = bacc.Bacc("TRN2", target_bir_lowering=False, debug=False,
                   num_devices=NCORE, num_swdge_queues=NQ)
    dt = nc.dram_tensor
    tbl = dt("tbl", [VT, 128], F16, kind="ExternalInput")
    tail_idx = dt("tail_idx", [128, NTAIL // 16], I16, kind="ExternalInput")
    glob_idx = dt("glob_idx", [128, NCHUNK, NGI // 16], I16,
                  kind="ExternalInput")
    wgt_in = dt("wgt", [1, NJ], F32, kind="ExternalInput")
    cbw16 = dt("cbw16", [128, 33], F16, kind="ExternalInput")
    att_b = dt("att_b", [WIN, 1], F32, kind="ExternalInput")
    att2_wT = dt("att2_wT", [D, LOCAL], F32, kind="ExternalInput")
    att2_b = dt("att2_b", [LOCAL, 1], F32, kind="ExternalInput")
    cw16 = dt("cw16", [128, 600], F16, kind="ExternalInput")
    convb = dt("convb", [NF, 3], F32, kind="ExternalInput")
    mf_pack = dt("mf_pack", [100, 300], F16, kind="ExternalInput")
    mf_b = dt("mf_b", [GOUT, 1], F32, kind="ExternalInput")
    fin_pack = dt("fin_pack", [201, 400], F16, kind="ExternalInput")
    f2_pack = dt("f2_pack", [400, NCLS], F16, kind="ExternalInput")
    f2_b = dt("f2_b", [BSH, NCLS], F32, kind="ExternalInput")
    y = dt("y", [BSH, NCLS], F32, kind="ExternalOutput")

    with tile.TileContext(nc) as tc:
        with tc.tile_pool(name="const", bufs=1) as cp, \
                tc.tile_pool(name="psA", bufs=1, space="PSUM") as psA, \
                tc.tile_pool(name="psC", bufs=2, space="PSUM") as psC, \
                tc.tile_pool(name="psD", bufs=2, space="PSUM") as psD, \
                tc.tile_pool(name="dram", bufs=2, space="DRAM") as dp:

            def cload(dram_t, shape, dtp):
                nm = f"c_{dram_t.name}"
                t = cp.tile(shape, dtp, tag=nm, name=nm)
                nc.sync.dma_start(t[:], dram_t[tuple(slice(0, s) for s in shape)])
                return t

            # warm-up collective to absorb CC-stack init latency
            wu_in = dp.tile([1, 8], F32)
            wu_out = dp.tile([NCORE, 8], F32)
            warm = cp.tile([1, 8], F32)
            nc.vector.memset(warm[:], 0.0)
            nc.gpsimd.dma_start(wu_in[:], warm[:])
            nc.gpsimd.collective_compute(
                "AllGather", ALU.bypass,
                replica_groups=[list(range(NCORE))],
                ins=[wu_in.opt()], outs=[wu_out.opt()])

            # tail-critical loads first, then the tail gather kicks off
            ti = cp.tile([128, NTAIL // 16], I16)
            nc.sync.dma_start(ti[:], tail_idx[:, :])
            cbw = cload(cbw16, [128, 33], F16)
            abt = cload(att_b, [WIN, 1], F32)
            ET = cp.tile([128, 1, NTAIL], F16)
            nc.gpsimd.dma_gather(out_ap=ET[:], in_ap=tbl[:, :], idxs_ap=ti[:],
                                 num_idxs=NTAIL, num_idxs_reg=NTAIL,
                                 elem_size=128, transpose=True, queue_num=0)
            gi = cp.tile([128, NCHUNK, NGI // 16], I16)
            nc.sync.dma_start(gi[:], glob_idx[:, :, :])
            ones5 = cp.tile([WIN, 1], F16)
            nc.vector.memset(ones5[:], 1.0)
            ones_k1 = cp.tile([1, D], F32)
            nc.vector.memset(ones_k1[:], 1.0)

            with tc.tile_wait_until(0.02):
                wgt = cload(wgt_in, [1, NJ], F32)
                a2w = cload(att2_wT, [D, LOCAL], F32)
                a2b = cload(att2_b, [LOCAL, 1], F32)
                cw = cload(cw16, [128, 600], F16)
                cb = cload(convb, [NF, 3], F32)
                mfw = cload(mf_pack, [100, 300], F16)
                mfb = cload(mf_b, [GOUT, 1], F32)
                fk0 = cp.tile([100, 400], F16)
                nc.sync.dma_start(fk0[:], fin_pack[0:100, :])
                fk1 = cp.tile([101, 400], F16)
                nc.sync.dma_start(fk1[:], fin_pack[100:201, :])
                f2t = []
                for m in range(4):
                    t = cp.tile([100, NCLS], F16, tag=f"f2_{m}", name=f"f2sb{m}")
                    nc.sync.dma_start(t[:], f2_pack[100 * m:100 * (m + 1), :])
                    f2t.append(t)
                f2bt = cload(f2_b, [BSH, NCLS], F32)

            # scores (rows 0:5) + per-token embedding sums (row 32)
            ps = psA.tile([33, NTAIL], F32, tag="sc")
            nc.tensor.matmul(ps[:], cbw[:], ET[:, 0, :])
            xs = cp.tile([WIN, NTAIL], F32)
            nc.scalar.activation(xs[:], ps[0:WIN, :], AF.Identity, bias=abt[:])
            ss = cp.tile([1, NTAIL], F32)
            nc.scalar.copy(ss[0:1, :], ps[32:33, :])
            # tanh(x) ~ x - x^3/3  (|x| <= ~0.02)
            x2 = cp.tile([WIN, NTAIL], F32)
            nc.vector.tensor_mul(x2[:], xs[:], xs[:])
            nc.vector.tensor_scalar(x2[:], x2[:], -1.0 / 3.0, 1.0,
                                    ALU.mult, ALU.add)
            xst = cp.tile([WIN, NTAIL], F16)
            nc.vector.tensor_mul(xst[:], xs[:], x2[:])
            pa = psA.tile([1, NTAIL], F32, tag="sc")
            nc.tensor.matmul(pa[:], ones5[:], xst[:])
            asum = cp.tile([1, NTAIL], F32)
            nc.scalar.copy(asum[0:1, :], pa[:])
            nasum = cp.tile([1, NTAIL], F32)
            nc.vector.tensor_scalar_mul(nasum[:], asum[:], -1.0)
            # per-block batch max (cols 0:TBLK) and -min (cols TBLK:) of asum
            partial = cp.tile([1, 2 * TBLK], F32)
            nc.vector.reduce_max(
                partial[0:1, 0:TBLK],
                asum[0:1, :].rearrange("p (a b) -> p a b", b=BSH), axis=AX.X)
            nc.vector.reduce_max(
                partial[0:1, TBLK:2 * TBLK],
                nasum[0:1, :].rearrange("p (a b) -> p a b", b=BSH), axis=AX.X)
            cc_in = dp.tile([1, 2 * TBLK], F32)
            cc_out = dp.tile([NCORE, 2 * TBLK], F32)
            nc.sync.dma_start(cc_in[:], partial[:])

            # ---- global CNN path (overlaps the AllGather) ----
            EG = cp.tile([128, BSH * TOKR], F16)
            pooled = [cp.tile([NF, BSH], F32, tag=f"pool{c}", name=f"pooled{c}")
                      for c in range(3)]
            taps = [(0, 1), (1, 2), (3, 3)]

            def emit_gather(j):
                sl = EG[:, j * NGI:(j + 1) * NGI]
                out_ap = AP(sl.tensor, sl.offset,
                            [list(sl.ap[0]), [1, 1], [1, NGI]])
                nc.gpsimd.dma_gather(
                    out_ap=out_ap, in_ap=tbl[:, :], idxs_ap=gi[:, j, :],
                    num_idxs=NGI, num_idxs_reg=NGI, elem_size=128,
                    transpose=True, queue_num=(1 + j) % NQ)

            def emit_convs(j):
                for c, (t0, ntap) in enumerate(taps):
                    T = LG - ntap + 1
                    pc = psC.tile([NF, GCH, 512], F32, tag="conv")
                    for rr in range(GCH):
                        c0 = (j * GCH + rr) * TOKR
                        for k in range(ntap):
                            nc.tensor.matmul(
                                pc[:, rr, 0:T],
                                cw[:, (t0 + k) * 100:(t0 + k + 1) * 100],
                                EG[:, c0 + k:c0 + k + T],
                                start=(k == 0), stop=(k == ntap - 1))
                    nc.vector.reduce_max(
                        pooled[c][:, j * GCH:(j + 1) * GCH],
                        pc[:, :, 0:T], axis=AX.X)

            with tc.tile_wait_until(0.025):
                for j in range(3):
                    emit_gather(j)
            with tc.tile_wait_until(0.03):
                nc.gpsimd.collective_compute(
                    "AllGather", ALU.bypass,
                    replica_groups=[list(range(NCORE))],
                    ins=[cc_in.opt()], outs=[cc_out.opt()])
            with tc.tile_wait_until(0.1):
                for j in range(3, NCHUNK):
                    emit_gather(j)
            with tc.tile_wait_until(0.15):
                for j in range(NCHUNK):
                    emit_convs(j)

            # ---- finish local path (after AllGather) ----
            tc.tile_set_cur_wait(0.3)
            gm = cp.tile([1, NCORE, 2 * TBLK], F32)
            nc.sync.dma_start(gm[:], cc_out[:, :].unsqueeze(0))
            gmax = cp.tile([1, 2 * TBLK], F32)
            nc.vector.reduce_max(gmax[:], gm[:].rearrange("p g t -> p t g"),
                                 axis=AX.X)
            wmax = cp.tile([1, NP_TAIL], F32)
            wneg = cp.tile([1, NP_TAIL], F32)
            nc.vector.reduce_max(wmax[:], _win_ap(gmax, 0, NP_TAIL), axis=AX.X)
            nc.vector.reduce_max(wneg[:], _win_ap(gmax, TBLK, NP_TAIL),
                                 axis=AX.X)
            cmp = cp.tile([1, NP_TAIL], F32)
            nc.vector.tensor_sub(cmp[:], wmax[:], wneg[:])
            nc.vector.tensor_scalar_mul(cmp[:], cmp[:], CR)
            judge = cp.tile([1, NJ], F32)
            nc.vector.tensor_tensor(
                judge[0:1, :].rearrange("p (a b) -> p a b", b=BSH),
                ss[0:1, JOFF:JOFF + NJ].rearrange("p (a b) -> p a b", b=BSH),
                cmp[0:1, :].unsqueeze(2).broadcast_to([1, NP_TAIL, BSH]),
                op=ALU.is_gt)
            nc.vector.tensor_mul(judge[:], judge[:], wgt[:])
            jb = psA.tile([D, NJ], F32, tag="sc")
            nc.tensor.matmul(jb[:], ones_k1[:], judge[0:1, :])
            sET = cp.tile([D, NJ], F32)
            nc.vector.tensor_tensor(sET[:], ET[0:D, 0, JOFF:JOFF + NJ], jb[:],
                                    op=ALU.mult)
            twT = cp.tile([D, BSH], F32)
            nc.vector.reduce_sum(
                twT[:], sET[:].rearrange("p (blk b) -> p b blk", b=BSH),
                axis=AX.X)
            lup = psD.tile([LOCAL, BSH], F32, tag="head")
            nc.tensor.matmul(lup[:], a2w[:], twT[:])
            luT = cp.tile([LOCAL, BSH], F16)
            nc.scalar.activation(luT[:], lup[:], AF.Identity, bias=a2b[:])

            # ---- head ----
            poolr = [cp.tile([NF, BSH], F16, tag=f"poolr{c}", name=f"poolr{c}")
                     for c in range(3)]
            for c in range(3):
                nc.scalar.activation(poolr[c][:], pooled[c][:], AF.Relu,
                                     bias=cb[:, c:c + 1])
            gup = psD.tile([GOUT, BSH], F32, tag="head")
            for c in range(3):
                nc.tensor.matmul(gup[:], mfw[:, 100 * c:100 * (c + 1)],
                                 poolr[c][:], start=(c == 0), stop=(c == 2))
            guT = cp.tile([GOUT + 1, BSH], F16)
            nc.vector.memset(guT[:], 1.0)
            nc.scalar.activation(guT[0:GOUT, :], gup[:], AF.Identity, bias=mfb[:])
            hT = [cp.tile([100, BSH], F16, tag=f"h{m}", name=f"hT{m}")
                  for m in range(4)]
            for m in range(4):
                hp = psD.tile([100, BSH], F32, tag="head")
                nc.tensor.matmul(hp[:], fk0[:, 100 * m:100 * (m + 1)], luT[:],
                                 start=True, stop=False)
                nc.tensor.matmul(hp[:], fk1[:, 100 * m:100 * (m + 1)], guT[:],
                                 start=False, stop=True)
                nc.scalar.activation(hT[m][:], hp[:], AF.Relu)
            out_sb = cp.tile([BSH, NCLS], F32)
            for n0, nn in N_TILES_OUT:
                op_ = psD.tile([BSH, 512], F32, tag="head")
                for m in range(4):
                    nc.tensor.matmul(op_[:, 0:nn], hT[m][:],
                                     f2t[m][:, n0:n0 + nn],
                                     start=(m == 0), stop=(m == 3))
                nc.vector.tensor_tensor(
                    out_sb[:, n0:n0 + nn], op_[:, 0:nn],
                    f2bt[:, n0:n0 + nn], op=ALU.add)
            nc.sync.dma_start(y[:, :], out_sb[:])

    nc.compile()
    return nc


def _prep(inputs):
    """host-side packing; returns per-core in_maps."""
    emb = np.asarray(inputs["emb"], np.float32)
    l_txt = np.asarray(inputs["l_train_text"])
    g_txt = np.asarray(inputs["g_train_text"])

    att_w = np.asarray(inputs["att_w"], np.float32)
    combo = np.zeros((128, 33), np.float32)
    combo[0:D, 0:WIN] = att_w.T
    combo[0:D, 32] = 1.0
    cwp = np.zeros((128, 600), np.float32)
    cwp[0:D, 0:100] = np.asarray(inputs["conv1_w"])[:, 0, 0, :].T
    cwp[0:D, 100:200] = np.asarray(inputs["conv2_w"])[:, 0, 0, :].T
    cwp[0:D, 200:300] = np.asarray(inputs["conv2_w"])[:, 0, 1, :].T
    cwp[0:D, 300:400] = np.asarray(inputs["conv3_w"])[:, 0, 0, :].T
    cwp[0:D, 400:500] = np.asarray(inputs["conv3_w"])[:, 0, 1, :].T
    cwp[0:D, 500:600] = np.asarray(inputs["conv3_w"])[:, 0, 2, :].T
    convb = np.stack([np.asarray(inputs["conv1_b"]),
                      np.asarray(inputs["conv2_b"]),
                      np.asarray(inputs["conv3_b"])], axis=1).astype(np.float32)
    mf_w = np.asarray(inputs["mf_w"], np.float32)
    mfp = np.zeros((100, 300), np.float32)
    for c in range(3):
        mfp[:, 100 * c:100 * (c + 1)] = mf_w[:, 100 * c:100 * (c + 1)].T
    fin_w = np.asarray(inputs["fin_w"], np.float32)
    finp = np.zeros((201, 400), np.float32)
    finp[0:200] = fin_w.T
    finp[200] = np.asarray(inputs["fin_b"], np.float32)
    f2p = np.asarray(inputs["fin2_w"], np.float32).T.astype(np.float16)
    f2b = np.asarray(inputs["fin2_b"], np.float32)[None, :]

    # tw weights: w_p = P^-(NP_TAIL-k), col = k*BSH + b
    wgt = np.zeros((1, NJ), np.float32)
    for k in range(NP_TAIL):
        wgt[0, k * BSH:(k + 1) * BSH] = np.float64(P) ** -(NP_TAIL - k)

    shared = {
        "wgt": wgt,
        "cbw16": combo.astype(np.float16),
        "att_b": np.asarray(inputs["att_b"], np.float32)[:, None],
        "att2_wT": np.asarray(inputs["att2_w"], np.float32).T.copy(),
        "att2_b": np.asarray(inputs["att2_b"], np.float32)[:, None],
        "cw16": cwp.astype(np.float16), "convb": convb,
        "mf_pack": mfp.astype(np.float16),
        "mf_b": np.asarray(inputs["mf_b"], np.float32)[:, None],
        "fin_pack": finp.astype(np.float16), "f2_pack": f2p,
        "f2_b": np.broadcast_to(f2b, (BSH, NCLS)).copy(),
    }

    in_maps = []
    for core in range(NCORE):
        ls = l_txt[core * BSH:(core + 1) * BSH]
        gs = g_txt[core * BSH:(core + 1) * BSH]
        # tail slots: col = blk*BSH + b, token t = P0 + blk
        blk = np.arange(NTAIL) // BSH
        bb = np.arange(NTAIL) % BSH
        ttok = ls[bb, P0 + blk].astype(np.int64)
        # global slots: per row, slot i -> token t=min(i, LG-1)
        tt = np.minimum(np.arange(TOKR), LG - 1)
        gtok = gs[:, tt].astype(np.int64)          # [BSH, TOKR]
        uniq = np.unique(np.concatenate([ttok, gtok.ravel()]))
        assert len(uniq) <= VT
        tblr = np.zeros((VT, 128), np.float16)
        tblr[:len(uniq), 0:D] = emb[uniq]
        tidx = _wrap_idx(np.searchsorted(uniq, ttok))
        gidx = np.zeros((128, NCHUNK, NGI // 16), np.int16)
        gr = np.searchsorted(uniq, gtok).reshape(NCHUNK, NGI)
        for j in range(NCHUNK):
            gidx[:, j, :] = _wrap_idx(gr[j])
        m = dict(shared)
        m["tbl"] = tblr
        m["tail_idx"] = tidx
        m["glob_idx"] = gidx
        in_maps.append(m)
    return in_maps


def _run(inputs, trace=False, tmpdir=None):
    if "nc" not in _CACHE:
        _CACHE["nc"] = _build()
    nc = _CACHE["nc"]
    in_maps = _prep(inputs)
    res = run_bass_kernel_spmd(nc, in_maps, list(range(NCORE)),
                               trace=trace, tmpdir=tmpdir)
    out = np.concatenate([res.results[i]["y"] for i in range(NCORE)], axis=0)
    return out, res


def kernel(**inputs):
    out, _ = _run(inputs, trace=False)
    return out


# revision 24
# speedup vs baseline: 1.0653x; 1.0653x over previous
"""Trainium2 Bass kernel for nn_CNN_Att_16887811408068.

Self-contained: hardcodes shapes/sharding. Data-parallel over batch on
8 NeuronCores. Each core gets a privately remapped embedding table
(its <=16K distinct tokens packed into [32768, 128] f16), so SWDGE
transpose-mode gathers (int16 indices, 256B elements) land embeddings
directly in [dims x tokens] layout -- no PE transposes, no parity
selects. Global rows are padded to 448 tokens and gathered two rows per
896-index gather (the SWDGE descriptor-ring maximum), round-robin over
2 SWDGE queues (queue parity == semaphore-lane parity keeps per-lane
completion FIFO). The batch min/max "comparison" needs only the last 4
positions at this tolerance; its tiny AllGather is issued right after
the tail scores so it overlaps the conv stream (tile_wait_until staging
keeps the scheduler from queueing it behind the gathers).
"""
import numpy as np

import concourse.bacc as bacc
import concourse.mybir as mybir
import concourse.tile as tile
from concourse.ap import AP
from concourse.bass_utils import run_bass_kernel_spmd

F32 = mybir.dt.float32
F16 = mybir.dt.float16
I16 = mybir.dt.int16
AF = mybir.ActivationFunctionType
ALU = mybir.AluOpType
AX = mybir.AxisListType

# problem constants
V, D, WIN, P, CR = 50000, 100, 5, 411, 0.8
LOCAL, NF, GOUT, NCLS = 100, 100, 100, 2987
B, LL, LG = 256, P + WIN - 1, 411
NCORE = 8
BSH = B // NCORE                     # 32 batch rows per core

# local tail: positions p in [P0, P); token range t in [P0, LL)
NP_TAIL = 4
P0 = P - NP_TAIL                     # 407
TBLK = LL - P0                       # 8 token blocks (t values)
NTAIL = TBLK * BSH                   # 256 tail token slots
JOFF = 2 * BSH                       # col offset of t'=P0+2 block
NJ = NP_TAIL * BSH                   # 128 judge cols

# global path
TOKR = 448                           # padded tokens per row
GCH = 2                              # rows per gather chunk
NGI = GCH * TOKR                     # 896 indices per gather
NCHUNK = BSH // GCH                  # 16 chunks
VT = 32768                           # remapped table rows
NQ = 2                               # SWDGE queues

N_TILES_OUT = [(i * 512, min(512, NCLS - i * 512))
               for i in range((NCLS + 511) // 512)]

_CACHE = {}


def _wrap_idx(vals):
    """int16 gather index layout: idx i at [i%16, i//16], replicated to
    all eight 16-partition groups -> [128, n//16]."""
    n = len(vals)
    g = np.zeros((16, n // 16), np.int16)
    g[np.arange(n) % 16, np.arange(n) // 16] = vals.astype(np.int16)
    return np.tile(g, (8, 1))


def _win_ap(t, col0, n):
    """overlapping window view [1, n, WIN] starting at free col col0."""
    base = t[0:1, col0:col0 + n]
    return AP(base.tensor, base.offset, [list(base.ap[0]), [1, n], [1, WIN]])


def _build():
    nc = bacc.Bacc("TRN2", target_bir_lowering=False, debug=False,
                   num_devices=NCORE, num_swdge_queues=NQ)
    dt = nc.dram_tensor
    tbl = dt("tbl", [VT, 128], F16, kind="ExternalInput")
    tail_idx = dt("tail_idx", [128, NTAIL // 16], I16, kind="ExternalInput")
    glob_idx = dt("glob_idx", [128, NCHUNK, NGI // 16], I16,
                  kind="ExternalInput")
    wgt_in = dt("wgt", [1, NJ], F32, kind="ExternalInput")
    cbw16 = dt("cbw16", [128, 33], F16, kind="ExternalInput")
    att_b = dt("att_b", [WIN, 1], F32, kind="ExternalInput")
    att2_wT = dt("att2_wT", [D, LOCAL], F32, kind="ExternalInput")
    att2_b = dt("att2_b", [LOCAL, 1], F32, kind="ExternalInput")
    cw16 = dt("cw16", [128, 600], F16, kind="ExternalInput")
    convb = dt("convb", [NF, 3], F32, kind="ExternalInput")
    mf_pack = dt("mf_pack", [100, 300], F16, kind="ExternalInput")
    mf_b = dt("mf_b", [GOUT, 1], F32, kind="ExternalInput")
    fin_pack = dt("fin_pack", [201, 400], F16, kind="ExternalInput")
    f2_pack = dt("f2_pack", [400, NCLS], F16, kind="ExternalInput")
    f2_b = dt("f2_b", [BSH, NCLS], F32, kind="ExternalInput")
    y = dt("y", [BSH, NCLS], F32, kind="ExternalOutput")

    with tile.TileContext(nc) as tc:
        with tc.tile_pool(name="const", bufs=1) as cp, \
                tc.tile_pool(name="psA", bufs=1, space="PSUM") as psA, \
                tc.tile_pool(name="psC", bufs=2, space="PSUM") as psC, \
                tc.tile_pool(name="psD", bufs=2, space="PSUM") as psD, \
                tc.tile_pool(name="dram", bufs=2, space="DRAM") as dp:

            def cload(dram_t, shape, dtp):
                nm = f"c_{dram_t.name}"
                t = cp.tile(shape, dtp, tag=nm, name=nm)
                nc.sync.dma_start(t[:], dram_t[tuple(slice(0, s) for s in shape)])
                return t

            # tail-critical loads first, then the tail gather kicks off
            ti = cp.tile([128, NTAIL // 16], I16)
            nc.sync.dma_start(ti[:], tail_idx[:, :])
            cbw = cload(cbw16, [128, 33], F16)
            abt = cload(att_b, [WIN, 1], F32)
            ET = cp.tile([128, 1, NTAIL], F16)
            nc.gpsimd.dma_gather(out_ap=ET[:], in_ap=tbl[:, :], idxs_ap=ti[:],
                                 num_idxs=NTAIL, num_idxs_reg=NTAIL,
                                 elem_size=128, transpose=True, queue_num=0)
            gi = cp.tile([128, NCHUNK, NGI // 16], I16)
            nc.sync.dma_start(gi[:], glob_idx[:, :, :])
            ones5 = cp.tile([WIN, 1], F16)
            nc.vector.memset(ones5[:], 1.0)
            ones_k1 = cp.tile([1, D], F32)
            nc.vector.memset(ones_k1[:], 1.0)

            with tc.tile_wait_until(0.02):
                wgt = cload(wgt_in, [1, NJ], F32)
                a2w = cload(att2_wT, [D, LOCAL], F32)
                a2b = cload(att2_b, [LOCAL, 1], F32)
                cw = cload(cw16, [128, 600], F16)
                cb = cload(convb, [NF, 3], F32)
                mfw = cload(mf_pack, [100, 300], F16)
                mfb = cload(mf_b, [GOUT, 1], F32)
                fk0 = cp.tile([100, 400], F16)
                nc.sync.dma_start(fk0[:], fin_pack[0:100, :])
                fk1 = cp.tile([101, 400], F16)
                nc.sync.dma_start(fk1[:], fin_pack[100:201, :])
                f2t = []
                for m in range(4):
                    t = cp.tile([100, NCLS], F16, tag=f"f2_{m}", name=f"f2sb{m}")
                    nc.sync.dma_start(t[:], f2_pack[100 * m:100 * (m + 1), :])
                    f2t.append(t)
                f2bt = cload(f2_b, [BSH, NCLS], F32)

            # scores (rows 0:5) + per-token embedding sums (row 32)
            ps = psA.tile([33, NTAIL], F32, tag="sc")
            nc.tensor.matmul(ps[:], cbw[:], ET[:, 0, :])
            xs = cp.tile([WIN, NTAIL], F32)
            nc.scalar.activation(xs[:], ps[0:WIN, :], AF.Identity, bias=abt[:])
            ss = cp.tile([1, NTAIL], F32)
            nc.scalar.copy(ss[0:1, :], ps[32:33, :])
            # tanh(x) ~ x - x^3/3  (|x| <= ~0.02)
            x2 = cp.tile([WIN, NTAIL], F32)
            nc.vector.tensor_mul(x2[:], xs[:], xs[:])
            nc.vector.tensor_scalar(x2[:], x2[:], -1.0 / 3.0, 1.0,
                                    ALU.mult, ALU.add)
            xst = cp.tile([WIN, NTAIL], F16)
            nc.vector.tensor_mul(xst[:], xs[:], x2[:])
            pa = psA.tile([1, NTAIL], F32, tag="sc")
            nc.tensor.matmul(pa[:], ones5[:], xst[:])
            asum = cp.tile([1, NTAIL], F32)
            nc.scalar.copy(asum[0:1, :], pa[:])
            nasum = cp.tile([1, NTAIL], F32)
            nc.vector.tensor_scalar_mul(nasum[:], asum[:], -1.0)
            # per-block batch max (cols 0:TBLK) and -min (cols TBLK:) of asum
            partial = cp.tile([1, 2 * TBLK], F32)
            nc.vector.reduce_max(
                partial[0:1, 0:TBLK],
                asum[0:1, :].rearrange("p (a b) -> p a b", b=BSH), axis=AX.X)
            nc.vector.reduce_max(
                partial[0:1, TBLK:2 * TBLK],
                nasum[0:1, :].rearrange("p (a b) -> p a b", b=BSH), axis=AX.X)
            cc_in = dp.tile([1, 2 * TBLK], F32)
            cc_out_t = nc.dram_tensor("cc_out_sh", [NCORE, 2 * TBLK], F32,
                                      addr_space="Shared")
            cc_out = cc_out_t[:, :]
            nc.sync.dma_start(cc_in[:], partial[:])

            # ---- global CNN path (overlaps the AllGather) ----
            EG = cp.tile([128, BSH * TOKR], F16)
            pooled = [cp.tile([NF, BSH], F32, tag=f"pool{c}", name=f"pooled{c}")
                      for c in range(3)]
            taps = [(0, 1), (1, 2), (3, 3)]

            def emit_gather(j):
                sl = EG[:, j * NGI:(j + 1) * NGI]
                out_ap = AP(sl.tensor, sl.offset,
                            [list(sl.ap[0]), [1, 1], [1, NGI]])
                nc.gpsimd.dma_gather(
                    out_ap=out_ap, in_ap=tbl[:, :], idxs_ap=gi[:, j, :],
                    num_idxs=NGI, num_idxs_reg=NGI, elem_size=128,
                    transpose=True, queue_num=(1 + j) % NQ)

            def emit_convs(j):
                for c, (t0, ntap) in enumerate(taps):
                    T = LG - ntap + 1
                    pc = psC.tile([NF, GCH, 512], F32, tag="conv")
                    for rr in range(GCH):
                        c0 = (j * GCH + rr) * TOKR
                        for k in range(ntap):
                            nc.tensor.matmul(
                                pc[:, rr, 0:T],
                                cw[:, (t0 + k) * 100:(t0 + k + 1) * 100],
                                EG[:, c0 + k:c0 + k + T],
                                start=(k == 0), stop=(k == ntap - 1))
                    nc.vector.reduce_max(
                        pooled[c][:, j * GCH:(j + 1) * GCH],
                        pc[:, :, 0:T], axis=AX.X)

            with tc.tile_wait_until(0.025):
                for j in range(3):
                    emit_gather(j)
            with tc.tile_wait_until(0.03):
                nc.gpsimd.collective_compute(
                    "AllGather", ALU.bypass,
                    replica_groups=[list(range(NCORE))],
                    ins=[cc_in.opt()], outs=[cc_out.opt()])
            with tc.tile_wait_until(0.1):
                for j in range(3, NCHUNK):
                    emit_gather(j)
            with tc.tile_wait_until(0.15):
                for j in range(NCHUNK):
                    emit_convs(j)

            # ---- finish local path (after AllGather) ----
            tc.tile_set_cur_wait(0.3)
            gm = cp.tile([1, NCORE, 2 * TBLK], F32)
            nc.sync.dma_start(gm[:], cc_out.unsqueeze(0))
            gmax = cp.tile([1, 2 * TBLK], F32)
            nc.vector.reduce_max(gmax[:], gm[:].rearrange("p g t -> p t g"),
                                 axis=AX.X)
            wmax = cp.tile([1, NP_TAIL], F32)
            wneg = cp.tile([1, NP_TAIL], F32)
            nc.vector.reduce_max(wmax[:], _win_ap(gmax, 0, NP_TAIL), axis=AX.X)
            nc.vector.reduce_max(wneg[:], _win_ap(gmax, TBLK, NP_TAIL),
                                 axis=AX.X)
            cmp = cp.tile([1, NP_TAIL], F32)
            nc.vector.tensor_sub(cmp[:], wmax[:], wneg[:])
            nc.vector.tensor_scalar_mul(cmp[:], cmp[:], CR)
            judge = cp.tile([1, NJ], F32)
            nc.vector.tensor_tensor(
                judge[0:1, :].rearrange("p (a b) -> p a b", b=BSH),
                ss[0:1, JOFF:JOFF + NJ].rearrange("p (a b) -> p a b", b=BSH),
                cmp[0:1, :].unsqueeze(2).broadcast_to([1, NP_TAIL, BSH]),
                op=ALU.is_gt)
            nc.vector.tensor_mul(judge[:], judge[:], wgt[:])
            jb = psA.tile([D, NJ], F32, tag="sc")
            nc.tensor.matmul(jb[:], ones_k1[:], judge[0:1, :])
            sET = cp.tile([D, NJ], F32)
            nc.vector.tensor_tensor(sET[:], ET[0:D, 0, JOFF:JOFF + NJ], jb[:],
                                    op=ALU.mult)
            twT = cp.tile([D, BSH], F32)
            nc.vector.reduce_sum(
                twT[:], sET[:].rearrange("p (blk b) -> p b blk", b=BSH),
                axis=AX.X)
            lup = psD.tile([LOCAL, BSH], F32, tag="head")
            nc.tensor.matmul(lup[:], a2w[:], twT[:])
            luT = cp.tile([LOCAL, BSH], F16)
            nc.scalar.activation(luT[:], lup[:], AF.Identity, bias=a2b[:])

            # ---- head ----
            poolr = [cp.tile([NF, BSH], F16, tag=f"poolr{c}", name=f"poolr{c}")
                     for c in range(3)]
            for c in range(3):
                nc.scalar.activation(poolr[c][:], pooled[c][:], AF.Relu,
                                     bias=cb[:, c:c + 1])
            gup = psD.tile([GOUT, BSH], F32, tag="head")
            for c in range(3):
                nc.tensor.matmul(gup[:], mfw[:, 100 * c:100 * (c + 1)],
                                 poolr[c][:], start=(c == 0), stop=(c == 2))
            guT = cp.tile([GOUT + 1, BSH], F16)
            nc.vector.memset(guT[:], 1.0)
            nc.scalar.activation(guT[0:GOUT, :], gup[:], AF.Identity, bias=mfb[:])
            hT = [cp.tile([100, BSH], F16, tag=f"h{m}", name=f"hT{m}")
                  for m in range(4)]
            for m in range(4):
                hp = psD.tile([100, BSH], F32, tag="head")
                nc.tensor.matmul(hp[:], fk0[:, 100 * m:100 * (m + 1)], luT[:],
                                 start=True, stop=False)
                nc.tensor.matmul(hp[:], fk1[:, 100 * m:100 * (m + 1)], guT[:],
                                 start=False, stop=True)
                nc.scalar.activation(hT[m][:], hp[:], AF.Relu)
            out_sb = cp.tile([BSH, NCLS], F32)
            for n0, nn in N_TILES_OUT:
                op_ = psD.tile([BSH, 512], F32, tag="head")
                for m in range(4):
                    nc.tensor.matmul(op_[:, 0:nn], hT[m][:],
                                     f2t[m][:, n0:n0 + nn],
                                     start=(m == 0), stop=(m == 3))
                nc.vector.tensor_tensor(
                    out_sb[:, n0:n0 + nn], op_[:, 0:nn],
                    f2bt[:, n0:n0 + nn], op=ALU.add)
            nc.sync.dma_start(y[:, :], out_sb[:])

    nc.compile()
    return nc


def _prep(inputs):
    """host-side packing; returns per-core in_maps."""
    emb = np.asarray(inputs["emb"], np.float32)
    l_txt = np.asarray(inputs["l_train_text"])
    g_txt = np.asarray(inputs["g_train_text"])

    att_w = np.asarray(inputs["att_w"], np.float32)
    combo = np.zeros((128, 33), np.float32)
    combo[0:D, 0:WIN] = att_w.T
    combo[0:D, 32] = 1.0
    cwp = np.zeros((128, 600), np.float32)
    cwp[0:D, 0:100] = np.asarray(inputs["conv1_w"])[:, 0, 0, :].T
    cwp[0:D, 100:200] = np.asarray(inputs["conv2_w"])[:, 0, 0, :].T
    cwp[0:D, 200:300] = np.asarray(inputs["conv2_w"])[:, 0, 1, :].T
    cwp[0:D, 300:400] = np.asarray(inputs["conv3_w"])[:, 0, 0, :].T
    cwp[0:D, 400:500] = np.asarray(inputs["conv3_w"])[:, 0, 1, :].T
    cwp[0:D, 500:600] = np.asarray(inputs["conv3_w"])[:, 0, 2, :].T
    convb = np.stack([np.asarray(inputs["conv1_b"]),
                      np.asarray(inputs["conv2_b"]),
                      np.asarray(inputs["conv3_b"])], axis=1).astype(np.float32)
    mf_w = np.asarray(inputs["mf_w"], np.float32)
    mfp = np.zeros((100, 300), np.float32)
    for c in range(3):
        mfp[:, 100 * c:100 * (c + 1)] = mf_w[:, 100 * c:100 * (c + 1)].T
    fin_w = np.asarray(inputs["fin_w"], np.float32)
    finp = np.zeros((201, 400), np.float32)
    finp[0:200] = fin_w.T
    finp[200] = np.asarray(inputs["fin_b"], np.float32)
    f2p = np.asarray(inputs["fin2_w"], np.float32).T.astype(np.float16)
    f2b = np.asarray(inputs["fin2_b"], np.float32)[None, :]

    # tw weights: w_p = P^-(NP_TAIL-k), col = k*BSH + b
    wgt = np.zeros((1, NJ), np.float32)
    for k in range(NP_TAIL):
        wgt[0, k * BSH:(k + 1) * BSH] = np.float64(P) ** -(NP_TAIL - k)

    shared = {
        "wgt": wgt,
        "cbw16": combo.astype(np.float16),
        "att_b": np.asarray(inputs["att_b"], np.float32)[:, None],
        "att2_wT": np.asarray(inputs["att2_w"], np.float32).T.copy(),
        "att2_b": np.asarray(inputs["att2_b"], np.float32)[:, None],
        "cw16": cwp.astype(np.float16), "convb": convb,
        "mf_pack": mfp.astype(np.float16),
        "mf_b": np.asarray(inputs["mf_b"], np.float32)[:, None],
        "fin_pack": finp.astype(np.float16), "f2_pack": f2p,
        "f2_b": np.broadcast_to(f2b, (BSH, NCLS)).copy(),
    }

    in_maps = []
    for core in range(NCORE):
        ls = l_txt[core * BSH:(core + 1) * BSH]
        gs = g_txt[core * BSH:(core + 1) * BSH]
        # tail slots: col = blk*BSH + b, token t = P0 + blk
        blk = np.arange(NTAIL) // BSH
        bb = np.arange(NTAIL) % BSH
        ttok = ls[bb, P0 + blk].astype(np.int64)
        # global slots: per row, slot i -> token t=min(i, LG-1)
        tt = np.minimum(np.arange(TOKR), LG - 1)
        gtok = gs[:, tt].astype(np.int64)          # [BSH, TOKR]
        uniq = np.unique(np.concatenate([ttok, gtok.ravel()]))
        assert len(uniq) <= VT
        tblr = np.zeros((VT, 128), np.float16)
        tblr[:len(uniq), 0:D] = emb[uniq]
        tidx = _wrap_idx(np.searchsorted(uniq, ttok))
        gidx = np.zeros((128, NCHUNK, NGI // 16), np.int16)
        gr = np.searchsorted(uniq, gtok).reshape(NCHUNK, NGI)
        for j in range(NCHUNK):
            gidx[:, j, :] = _wrap_idx(gr[j])
        m = dict(shared)
        m["tbl"] = tblr
        m["tail_idx"] = tidx
        m["glob_idx"] = gidx
        in_maps.append(m)
    return in_maps


def _run(inputs, trace=False, tmpdir=None):
    if "nc" not in _CACHE:
        _CACHE["nc"] = _build()
    nc = _CACHE["nc"]
    in_maps = _prep(inputs)
    res = run_bass_kernel_spmd(nc, in_maps, list(range(NCORE)),
                               trace=trace, tmpdir=tmpdir)
    out = np.concatenate([res.results[i]["y"] for i in range(NCORE)], axis=0)
    return out, res


def kernel(**inputs):
    out, _ = _run(inputs, trace=False)
    return out


# revision 25
# speedup vs baseline: 1.0955x; 1.0283x over previous
"""Trainium2 Bass kernel for nn_CNN_Att_16887811408068.

Self-contained: hardcodes shapes/sharding. Data-parallel over batch on
8 NeuronCores. Each core gets a privately remapped embedding table
(its <=16K distinct tokens packed into [32768, 128] f16), so SWDGE
transpose-mode gathers (int16 indices, 256B elements) land embeddings
directly in [dims x tokens] layout -- no PE transposes, no parity
selects. Global rows are padded to 448 tokens and gathered two rows per
896-index gather (the SWDGE descriptor-ring maximum), round-robin over
2 SWDGE queues (queue parity == semaphore-lane parity keeps per-lane
completion FIFO). The batch min/max "comparison" needs only the last 4
positions at this tolerance; its tiny AllGather is issued right after
the tail scores so it overlaps the conv stream (tile_wait_until staging
keeps the scheduler from queueing it behind the gathers).
"""
import numpy as np

import concourse.bacc as bacc
import concourse.mybir as mybir
import concourse.tile as tile
from concourse.ap import AP
from concourse.bass_utils import run_bass_kernel_spmd

F32 = mybir.dt.float32
F16 = mybir.dt.float16
I16 = mybir.dt.int16
AF = mybir.ActivationFunctionType
ALU = mybir.AluOpType
AX = mybir.AxisListType

# problem constants
V, D, WIN, P, CR = 50000, 100, 5, 411, 0.8
LOCAL, NF, GOUT, NCLS = 100, 100, 100, 2987
B, LL, LG = 256, P + WIN - 1, 411
NCORE = 8
BSH = B // NCORE                     # 32 batch rows per core

# local tail: positions p in [P0, P); token range t in [P0, LL)
NP_TAIL = 4
P0 = P - NP_TAIL                     # 407
TBLK = LL - P0                       # 8 token blocks (t values)
NTAIL = TBLK * BSH                   # 256 tail token slots
JOFF = 2 * BSH                       # col offset of t'=P0+2 block
NJ = NP_TAIL * BSH                   # 128 judge cols

# global path
TOKR = 448                           # padded tokens per row
GCH = 2                              # rows per gather chunk
NGI = GCH * TOKR                     # 896 indices per gather
NCHUNK = BSH // GCH                  # 16 chunks
VT = 32768                           # remapped table rows
NQ = 2                               # SWDGE queues

N_TILES_OUT = [(i * 512, min(512, NCLS - i * 512))
               for i in range((NCLS + 511) // 512)]

_CACHE = {}


def _wrap_idx(vals):
    """int16 gather index layout: idx i at [i%16, i//16], replicated to
    all eight 16-partition groups -> [128, n//16]."""
    n = len(vals)
    g = np.zeros((16, n // 16), np.int16)
    g[np.arange(n) % 16, np.arange(n) // 16] = vals.astype(np.int16)
    return np.tile(g, (8, 1))


def _win_ap(t, col0, n):
    """overlapping window view [1, n, WIN] starting at free col col0."""
    base = t[0:1, col0:col0 + n]
    return AP(base.tensor, base.offset, [list(base.ap[0]), [1, n], [1, WIN]])


def _build():
    nc = bacc.Bacc("TRN2", target_bir_lowering=False, debug=False,
                   num_devices=NCORE, num_swdge_queues=NQ)
    dt = nc.dram_tensor
    tbl = dt("tbl", [VT, 128], F16, kind="ExternalInput")
    tail_idx = dt("tail_idx", [128, NTAIL // 16], I16, kind="ExternalInput")
    glob_idx = dt("glob_idx", [128, NCHUNK, NGI // 16], I16,
                  kind="ExternalInput")
    wgt_in = dt("wgt", [1, NJ], F32, kind="ExternalInput")
    cbw16 = dt("cbw16", [128, 33], F16, kind="ExternalInput")
    att_b = dt("att_b", [WIN, 1], F32, kind="ExternalInput")
    att2_wT = dt("att2_wT", [D, LOCAL], F32, kind="ExternalInput")
    att2_b = dt("att2_b", [LOCAL, 1], F32, kind="ExternalInput")
    cw16 = dt("cw16", [128, 600], F16, kind="ExternalInput")
    convb = dt("convb", [NF, 3], F32, kind="ExternalInput")
    mf_pack = dt("mf_pack", [100, 300], F16, kind="ExternalInput")
    mf_b = dt("mf_b", [GOUT, 1], F32, kind="ExternalInput")
    fin_pack = dt("fin_pack", [201, 400], F16, kind="ExternalInput")
    f2_pack = dt("f2_pack", [400, NCLS], F16, kind="ExternalInput")
    f2_b = dt("f2_b", [BSH, NCLS], F32, kind="ExternalInput")
    y = dt("y", [BSH, NCLS], F32, kind="ExternalOutput")

    with tile.TileContext(nc) as tc:
        with tc.tile_pool(name="const", bufs=1) as cp, \
                tc.tile_pool(name="psA", bufs=1, space="PSUM") as psA, \
                tc.tile_pool(name="psC", bufs=2, space="PSUM") as psC, \
                tc.tile_pool(name="psD", bufs=2, space="PSUM") as psD, \
                tc.tile_pool(name="dram", bufs=2, space="DRAM") as dp:

            def cload(dram_t, shape, dtp):
                nm = f"c_{dram_t.name}"
                t = cp.tile(shape, dtp, tag=nm, name=nm)
                nc.sync.dma_start(t[:], dram_t[tuple(slice(0, s) for s in shape)])
                return t

            # tail-critical loads first, then the tail gather kicks off
            ti = cp.tile([128, NTAIL // 16], I16)
            nc.sync.dma_start(ti[:], tail_idx[:, :])
            cbw = cload(cbw16, [128, 33], F16)
            abt = cload(att_b, [WIN, 1], F32)
            ET = cp.tile([128, 1, NTAIL], F16)
            nc.gpsimd.dma_gather(out_ap=ET[:], in_ap=tbl[:, :], idxs_ap=ti[:],
                                 num_idxs=NTAIL, num_idxs_reg=NTAIL,
                                 elem_size=128, transpose=True, queue_num=0)
            gi = cp.tile([128, NCHUNK, NGI // 16], I16)
            nc.sync.dma_start(gi[:], glob_idx[:, :, :])
            ones5 = cp.tile([WIN, 1], F16)
            nc.vector.memset(ones5[:], 1.0)
            ones_k1 = cp.tile([1, D], F32)
            nc.vector.memset(ones_k1[:], 1.0)

            with tc.tile_wait_until(0.02):
                wgt = cload(wgt_in, [1, NJ], F32)
                a2w = cload(att2_wT, [D, LOCAL], F32)
                a2b = cload(att2_b, [LOCAL, 1], F32)
                cw = cload(cw16, [128, 600], F16)
                cb = cload(convb, [NF, 3], F32)
                mfw = cload(mf_pack, [100, 300], F16)
                mfb = cload(mf_b, [GOUT, 1], F32)
                fk0 = cp.tile([100, 400], F16)
                nc.sync.dma_start(fk0[:], fin_pack[0:100, :])
                fk1 = cp.tile([101, 400], F16)
                nc.sync.dma_start(fk1[:], fin_pack[100:201, :])
                f2t = []
                for m in range(4):
                    t = cp.tile([100, NCLS], F16, tag=f"f2_{m}", name=f"f2sb{m}")
                    nc.sync.dma_start(t[:], f2_pack[100 * m:100 * (m + 1), :])
                    f2t.append(t)
                f2bt = cload(f2_b, [BSH, NCLS], F32)

            # scores (rows 0:5) + per-token embedding sums (row 32)
            ps = psA.tile([33, NTAIL], F32, tag="sc")
            nc.tensor.matmul(ps[:], cbw[:], ET[:, 0, :])
            xs = cp.tile([WIN, NTAIL], F32)
            nc.scalar.activation(xs[:], ps[0:WIN, :], AF.Identity, bias=abt[:])
            ss = cp.tile([1, NTAIL], F32)
            nc.scalar.copy(ss[0:1, :], ps[32:33, :])
            # tanh(x) ~ x - x^3/3  (|x| <= ~0.02)
            x2 = cp.tile([WIN, NTAIL], F32)
            nc.vector.tensor_mul(x2[:], xs[:], xs[:])
            nc.vector.tensor_scalar(x2[:], x2[:], -1.0 / 3.0, 1.0,
                                    ALU.mult, ALU.add)
            xst = cp.tile([WIN, NTAIL], F16)
            nc.vector.tensor_mul(xst[:], xs[:], x2[:])
            pa = psA.tile([1, NTAIL], F32, tag="sc")
            nc.tensor.matmul(pa[:], ones5[:], xst[:])
            nasum = cp.tile([1, NTAIL], F32)
            nc.vector.tensor_scalar_mul(nasum[:], pa[:], -1.0)
            # per-block batch max (cols 0:TBLK) and -min (cols TBLK:) of asum
            partial = cp.tile([1, 2 * TBLK], F32)
            nc.vector.reduce_max(
                partial[0:1, 0:TBLK],
                pa[0:1, :].rearrange("p (a b) -> p a b", b=BSH), axis=AX.X)
            nc.vector.reduce_max(
                partial[0:1, TBLK:2 * TBLK],
                nasum[0:1, :].rearrange("p (a b) -> p a b", b=BSH), axis=AX.X)
            cc_in = dp.tile([1, 2 * TBLK], F32)
            cc_out_t = nc.dram_tensor("cc_out_sh", [NCORE, 2 * TBLK], F32,
                                      addr_space="Shared")
            cc_out = cc_out_t[:, :]
            nc.sync.dma_start(cc_in[:], partial[:])

            # ---- global CNN path (overlaps the AllGather) ----
            EG = cp.tile([128, BSH * TOKR], F16)
            pooled = [cp.tile([NF, BSH], F32, tag=f"pool{c}", name=f"pooled{c}")
                      for c in range(3)]
            taps = [(0, 1), (1, 2), (3, 3)]

            def emit_gather(j):
                sl = EG[:, j * NGI:(j + 1) * NGI]
                out_ap = AP(sl.tensor, sl.offset,
                            [list(sl.ap[0]), [1, 1], [1, NGI]])
                nc.gpsimd.dma_gather(
                    out_ap=out_ap, in_ap=tbl[:, :], idxs_ap=gi[:, j, :],
                    num_idxs=NGI, num_idxs_reg=NGI, elem_size=128,
                    transpose=True, queue_num=(1 + j) % NQ)

            def emit_convs(j):
                for c, (t0, ntap) in enumerate(taps):
                    T = LG - ntap + 1
                    pc = psC.tile([NF, GCH, 512], F32, tag="conv")
                    for rr in range(GCH):
                        c0 = (j * GCH + rr) * TOKR
                        for k in range(ntap):
                            nc.tensor.matmul(
                                pc[:, rr, 0:T],
                                cw[:, (t0 + k) * 100:(t0 + k + 1) * 100],
                                EG[:, c0 + k:c0 + k + T],
                                start=(k == 0), stop=(k == ntap - 1))
                    nc.vector.reduce_max(
                        pooled[c][:, j * GCH:(j + 1) * GCH],
                        pc[:, :, 0:T], axis=AX.X)

            with tc.tile_wait_until(0.025):
                for j in range(3):
                    emit_gather(j)
            with tc.tile_wait_until(0.03):
                nc.gpsimd.collective_compute(
                    "AllGather", ALU.bypass,
                    replica_groups=[list(range(NCORE))],
                    ins=[cc_in.opt()], outs=[cc_out.opt()])
            with tc.tile_wait_until(0.1):
                for j in range(3, NCHUNK):
                    emit_gather(j)
            with tc.tile_wait_until(0.15):
                for j in range(NCHUNK):
                    emit_convs(j)

            # ---- finish local path (after AllGather) ----
            tc.tile_set_cur_wait(0.3)
            gm = cp.tile([1, NCORE, 2 * TBLK], F32)
            nc.sync.dma_start(gm[:], cc_out.unsqueeze(0))
            gmax = cp.tile([1, 2 * TBLK], F32)
            nc.vector.reduce_max(gmax[:], gm[:].rearrange("p g t -> p t g"),
                                 axis=AX.X)
            wmax = cp.tile([1, NP_TAIL], F32)
            wneg = cp.tile([1, NP_TAIL], F32)
            nc.vector.reduce_max(wmax[:], _win_ap(gmax, 0, NP_TAIL), axis=AX.X)
            nc.vector.reduce_max(wneg[:], _win_ap(gmax, TBLK, NP_TAIL),
                                 axis=AX.X)
            cmp = cp.tile([1, NP_TAIL], F32)
            nc.vector.tensor_sub(cmp[:], wmax[:], wneg[:])
            nc.vector.tensor_scalar_mul(cmp[:], cmp[:], CR)
            judge = cp.tile([1, NJ], F32)
            nc.vector.tensor_tensor(
                judge[0:1, :].rearrange("p (a b) -> p a b", b=BSH),
                ss[0:1, JOFF:JOFF + NJ].rearrange("p (a b) -> p a b", b=BSH),
                cmp[0:1, :].unsqueeze(2).broadcast_to([1, NP_TAIL, BSH]),
                op=ALU.is_gt)
            nc.vector.tensor_mul(judge[:], judge[:], wgt[:])
            jb = psA.tile([D, NJ], F32, tag="sc")
            nc.tensor.matmul(jb[:], ones_k1[:], judge[0:1, :])
            sET = cp.tile([D, NJ], F32)
            nc.vector.tensor_tensor(sET[:], ET[0:D, 0, JOFF:JOFF + NJ], jb[:],
                                    op=ALU.mult)
            twT = cp.tile([D, BSH], F32)
            nc.vector.reduce_sum(
                twT[:], sET[:].rearrange("p (blk b) -> p b blk", b=BSH),
                axis=AX.X)
            lup = psD.tile([LOCAL, BSH], F32, tag="head")
            nc.tensor.matmul(lup[:], a2w[:], twT[:])
            luT = cp.tile([LOCAL, BSH], F16)
            nc.scalar.activation(luT[:], lup[:], AF.Identity, bias=a2b[:])

            # ---- head ----
            poolr = [cp.tile([NF, BSH], F16, tag=f"poolr{c}", name=f"poolr{c}")
                     for c in range(3)]
            for c in range(3):
                nc.scalar.activation(poolr[c][:], pooled[c][:], AF.Relu,
                                     bias=cb[:, c:c + 1])
            gup = psD.tile([GOUT, BSH], F32, tag="head")
            for c in range(3):
                nc.tensor.matmul(gup[:], mfw[:, 100 * c:100 * (c + 1)],
                                 poolr[c][:], start=(c == 0), stop=(c == 2))
            guT = cp.tile([GOUT + 1, BSH], F16)
            nc.vector.memset(guT[:], 1.0)
            nc.scalar.activation(guT[0:GOUT, :], gup[:], AF.Identity, bias=mfb[:])
            hT = [cp.tile([100, BSH], F16, tag=f"h{m}", name=f"hT{m}")
                  for m in range(4)]
            for m in range(4):
                hp = psD.tile([100, BSH], F32, tag="head")
                nc.tensor.matmul(hp[:], fk0[:, 100 * m:100 * (m + 1)], luT[:],
                                 start=True, stop=False)
                nc.tensor.matmul(hp[:], fk1[:, 100 * m:100 * (m + 1)], guT[:],
                                 start=False, stop=True)
                nc.scalar.activation(hT[m][:], hp[:], AF.Relu)
            out_sb = cp.tile([BSH, NCLS], F32)
            for n0, nn in N_TILES_OUT:
                op_ = psD.tile([BSH, 512], F32, tag="head")
                for m in range(4):
                    nc.tensor.matmul(op_[:, 0:nn], hT[m][:],
                                     f2t[m][:, n0:n0 + nn],
                                     start=(m == 0), stop=(m == 3))
                nc.vector.tensor_tensor(
                    out_sb[:, n0:n0 + nn], op_[:, 0:nn],
                    f2bt[:, n0:n0 + nn], op=ALU.add)
            nc.sync.dma_start(y[:, :], out_sb[:])

    nc.compile()
    return nc


def _prep(inputs):
    """host-side packing; returns per-core in_maps."""
    emb = np.asarray(inputs["emb"], np.float32)
    l_txt = np.asarray(inputs["l_train_text"])
    g_txt = np.asarray(inputs["g_train_text"])

    att_w = np.asarray(inputs["att_w"], np.float32)
    combo = np.zeros((128, 33), np.float32)
    combo[0:D, 0:WIN] = att_w.T
    combo[0:D, 32] = 1.0
    cwp = np.zeros((128, 600), np.float32)
    cwp[0:D, 0:100] = np.asarray(inputs["conv1_w"])[:, 0, 0, :].T
    cwp[0:D, 100:200] = np.asarray(inputs["conv2_w"])[:, 0, 0, :].T
    cwp[0:D, 200:300] = np.asarray(inputs["conv2_w"])[:, 0, 1, :].T
    cwp[0:D, 300:400] = np.asarray(inputs["conv3_w"])[:, 0, 0, :].T
    cwp[0:D, 400:500] = np.asarray(inputs["conv3_w"])[:, 0, 1, :].T
    cwp[0:D, 500:600] = np.asarray(inputs["conv3_w"])[:, 0, 2, :].T
    convb = np.stack([np.asarray(inputs["conv1_b"]),
                      np.asarray(inputs["conv2_b"]),
                      np.asarray(inputs["conv3_b"])], axis=1).astype(np.float32)
    mf_w = np.asarray(inputs["mf_w"], np.float32)
    mfp = np.zeros((100, 300), np.float32)
    for c in range(3):
        mfp[:, 100 * c:100 * (c + 1)] = mf_w[:, 100 * c:100 * (c + 1)].T
    fin_w = np.asarray(inputs["fin_w"], np.float32)
    finp = np.zeros((201, 400), np.float32)
    finp[0:200] = fin_w.T
    finp[200] = np.asarray(inputs["fin_b"], np.float32)
    f2p = np.asarray(inputs["fin2_w"], np.float32).T.astype(np.float16)
    f2b = np.asarray(inputs["fin2_b"], np.float32)[None, :]

    # tw weights: w_p = P^-(NP_TAIL-k), col = k*BSH + b
    wgt = np.zeros((1, NJ), np.float32)
    for k in range(NP_TAIL):
        wgt[0, k * BSH:(k + 1) * BSH] = np.float64(P) ** -(NP_TAIL - k)

    shared = {
        "wgt": wgt,
        "cbw16": combo.astype(np.float16),
        "att_b": np.asarray(inputs["att_b"], np.float32)[:, None],
        "att2_wT": np.asarray(inputs["att2_w"], np.float32).T.copy(),
        "att2_b": np.asarray(inputs["att2_b"], np.float32)[:, None],
        "cw16": cwp.astype(np.float16), "convb": convb,
        "mf_pack": mfp.astype(np.float16),
        "mf_b": np.asarray(inputs["mf_b"], np.float32)[:, None],
        "fin_pack": finp.astype(np.float16), "f2_pack": f2p,
        "f2_b": np.broadcast_to(f2b, (BSH, NCLS)).copy(),
    }

    in_maps = []
    for core in range(NCORE):
        ls = l_txt[core * BSH:(core + 1) * BSH]
        gs = g_txt[core * BSH:(core + 1) * BSH]
        # tail slots: col = blk*BSH + b, token t = P0 + blk
        blk = np.arange(NTAIL) // BSH
        bb = np.arange(NTAIL) % BSH
        ttok = ls[bb, P0 + blk].astype(np.int64)
        # global slots: per row, slot i -> token t=min(i, LG-1)
        tt = np.minimum(np.arange(TOKR), LG - 1)
        gtok = gs[:, tt].astype(np.int64)          # [BSH, TOKR]
        uniq = np.unique(np.concatenate([ttok, gtok.ravel()]))
        assert len(uniq) <= VT
        tblr = np.zeros((VT, 128), np.float16)
        tblr[:len(uniq), 0:D] = emb[uniq]
        tidx = _wrap_idx(np.searchsorted(uniq, ttok))
        gidx = np.zeros((128, NCHUNK, NGI // 16), np.int16)
        gr = np.searchsorted(uniq, gtok).reshape(NCHUNK, NGI)
        for j in range(NCHUNK):
            gidx[:, j, :] = _wrap_idx(gr[j])
        m = dict(shared)
        m["tbl"] = tblr
        m["tail_idx"] = tidx
        m["glob_idx"] = gidx
        in_maps.append(m)
    return in_maps


def _run(inputs, trace=False, tmpdir=None):
    if "nc" not in _CACHE:
        _CACHE["nc"] = _build()
    nc = _CACHE["nc"]
    in_maps = _prep(inputs)
    res = run_bass_kernel_spmd(nc, in_maps, list(range(NCORE)),
                               trace=trace, tmpdir=tmpdir)
    out = np.concatenate([res.results[i]["y"] for i in range(NCORE)], axis=0)
    return out, res


def kernel(**inputs):
    out, _ = _run(inputs, trace=False)
    return out
